# revision 1
# baseline (speedup 1.0000x reference)
"""GCNConv (PyG semantics: normalize=True, add_self_loops=True, edge_weight)
as a Trainium2 Bass kernel, SPMD over 8 NeuronCores.

Strategy: shard destination nodes across the 8 cores (per sharding hint).
The normalized adjacency (with self loops), A[dst, src] = dinv[src]*w*dinv[dst],
is materialized on the host as a dense bf16 matrix, pre-transposed and
pre-swizzled into the exact SBUF tile layout each core consumes. On device,
each core computes  agg = A_shard @ x  as a dense TensorEngine sweep:
x lives SBUF-resident in bf16 ([128, 80, 512], k-major), and per 128-dst
block the PE accumulates 80 k-tile matmuls into PSUM while the next block's
A^T stripe streams from DRAM. The transform  out = agg @ W + b  is software-
pipelined one block behind the sweep: PE transposes agg (4x 128x128 fp32),
ACT rounds the slices to f32r, the W matmuls run in f32r at full PE rate
(fp32 would cost 4 cycles/row), and DVE adds the bias on the PSUM->SBUF
copy. Results accumulate in one SBUF tile and leave in two DMAs.

The adjacency build (degree normalization folded into matrix values) is
host-side index preprocessing; every O(N*N*D) / O(N*D^2) FLOP runs on device.
A dense sweep is deliberate: the fast SWDGE gather ucode is unavailable on
this runtime and per-row indirect DMA measures ~17 GB/s, while the PE sweep
sustains the full array throughput.
"""
from contextlib import ExitStack

import numpy as np
import ml_dtypes

import concourse.bacc as bacc
import concourse.mybir as mybir
import concourse.tile as tile
from concourse.bass_utils import run_bass_kernel_spmd

P = 128
CORES = 8
BF16 = mybir.dt.bfloat16
F32 = mybir.dt.float32
F32R = mybir.dt.float32r


def _preprocess(x, edge_index, edge_attr):
    """Self loops, symmetric normalization, and the per-core dense A^T
    stripes in SBUF-swizzled layout: at_sw[c][g, p, k*P+m] = A^T[k*P+p (src),
    c*BPC*P + g*P + m (dst)]."""
    n = x.shape[0]
    src = np.asarray(edge_index[0], np.int64)
    dst = np.asarray(edge_index[1], np.int64)
    loop = np.arange(n, dtype=np.int64)
    src_f = np.concatenate([src, loop])
    dst_f = np.concatenate([dst, loop])
    ew = np.concatenate(
        [np.asarray(edge_attr, np.float64), np.ones(n, np.float64)])

    deg = np.zeros(n, np.float64)
    np.add.at(deg, dst_f, ew)
    dinv = np.where(deg > 0, 1.0 / np.sqrt(np.maximum(deg, 1e-300)), 0.0)
    sc = (dinv[src_f] * ew * dinv[dst_f]).astype(np.float32)

    bpc = -(-n // (CORES * P))           # dst blocks per core
    npad = CORES * bpc * P               # padded dst slot count
    kt = -(-n // P)                      # k-tiles over (padded) src nodes

    core_of = dst_f // (bpc * P)
    g_of = (dst_f % (bpc * P)) // P
    m_of = dst_f % P
    p_of = src_f % P
    col_of = (src_f // P) * P + m_of

    at_sw = np.zeros((CORES, bpc, P, kt * P), np.float32)
    np.add.at(at_sw, (core_of, g_of, p_of, col_of), sc)
    at_sw = at_sw.astype(ml_dtypes.bfloat16)

    return dict(bpc=bpc, npad=npad, kt=kt, at_sw=at_sw)


def _build_module(n, d_in, d_out, bpc, kt, reps=1):
    """Emit the SPMD per-core Bass program."""
    assert d_in % P == 0 and d_out % P == 0
    kt_w = d_in // P

    nc = bacc.Bacc("TRN2", target_bir_lowering=False, debug=False)
    x_d = nc.dram_tensor("x", [P, kt * d_in], BF16, kind="ExternalInput")
    at_d = nc.dram_tensor("at", [bpc, P, kt * P], BF16, kind="ExternalInput")
    W_d = nc.dram_tensor("W", [d_in, d_out], F32R, kind="ExternalInput")
    bias_d = nc.dram_tensor("bias", [P, d_out], F32, kind="ExternalInput")
    ident_d = nc.dram_tensor("ident", [P, P], F32, kind="ExternalInput")
    out_d = nc.dram_tensor("out", [bpc, P, d_out], F32, kind="ExternalOutput")

    with tile.TileContext(nc) as tc, ExitStack() as ctx:
        const = ctx.enter_context(tc.tile_pool(name="const", bufs=1))
        atp = ctx.enter_context(tc.tile_pool(name="atp", bufs=3))
        apool = ctx.enter_context(tc.tile_pool(name="aggsb", bufs=2))
        tpool = ctx.enter_context(tc.tile_pool(name="atsb", bufs=8))
        ps_agg = ctx.enter_context(tc.tile_pool(name="ps_agg", bufs=2, space="PSUM"))
        ps_t = ctx.enter_context(tc.tile_pool(name="ps_t", bufs=4, space="PSUM"))
        ps_out = ctx.enter_context(tc.tile_pool(name="ps_out", bufs=2, space="PSUM"))

        def load_at(g):
            t = atp.tile([P, kt, P], BF16, tag="at")
            nc.sync.dma_start(t[:], at_d[g].rearrange("p (k m) -> p k m", m=P))
            return t

        # prime the SP ring with the first two A^T stripes, then stream the
        # pre-swizzled x in 8 chunks (first 6 on the ACT ring, last 2 on SP)
        # so block 0's k-consumption tracks chunk arrivals on both rings
        at_q = [load_at(0), load_at(1)]
        x_sb = const.tile([P, kt, d_in], BF16)
        x_view = x_d.ap().rearrange("p (k d) -> p k d", d=d_in)
        k_step = -(-kt // 8)
        chunks = [(k0, min(kt, k0 + k_step)) for k0 in range(0, kt, k_step)]
        for i, (k0, k1) in enumerate(chunks):
            eng = nc.scalar if i < len(chunks) - 2 else nc.sync
            eng.dma_start(x_sb[:, k0:k1, :], x_view[:, k0:k1, :])
        W_sb = const.tile([P, kt_w, d_out], F32R)
        nc.scalar.dma_start(W_sb[:], W_d.ap().rearrange("(k p) d -> p k d", p=P))
        bias_sb = const.tile([P, d_out], F32)
        nc.scalar.dma_start(bias_sb[:], bias_d[:, :])
        ident_sb = const.tile([P, P], F32)
        nc.scalar.dma_start(ident_sb[:], ident_d[:, :])
        out_acc = const.tile([P, bpc, d_out], F32)

        def transform(g, agg_ps):
            # agg_ps [P dst, d_in] fp32 PSUM -> out_acc[:, g, :] = agg @ W + b.
            # PE transposes agg (fp32), ACT copies round to f32r, and the
            # transform matmuls run at full PE rate in f32r.
            agg_sb = apool.tile([P, d_in], F32, tag="agg")
            nc.scalar.copy(agg_sb[:], agg_ps[:])
            out_ps = ps_out.tile([P, d_out], F32)
            for ki in range(kt_w):
                pt = ps_t.tile([P, P], F32, tag="pt")
                nc.tensor.transpose(pt[:], agg_sb[:, ki * P:(ki + 1) * P],
                                    ident_sb[:])
                aT = tpool.tile([P, P], F32R, tag="aT")
                nc.scalar.copy(aT[:], pt[:])
                nc.tensor.matmul(out_ps[:], aT[:], W_sb[:, ki, :],
                                 start=(ki == 0), stop=(ki == kt_w - 1))
            nc.vector.tensor_add(out_acc[:, g, :], out_ps[:], bias_sb[:])

        order = [g for _ in range(reps) for g in range(bpc)]
        next_i = len(at_q)          # stripes 0,1 already in flight
        pending = None              # (g, agg_ps) awaiting transform
        for i, g in enumerate(order):
            at_sb = at_q.pop(0)
            if next_i < len(order):
                at_q.append(load_at(order[next_i]))
                next_i += 1
            agg_ps = ps_agg.tile([P, d_in], F32)
            for k in range(kt):
                nc.tensor.matmul(agg_ps[:], at_sb[:, k, :], x_sb[:, k, :],
                                 start=(k == 0), stop=(k == kt - 1))
            if pending is not None:
                transform(*pending)
            pending = (g, agg_ps)
            if g == bpc // 2 and g > 0:
                nc.scalar.dma_start(
                    out_d.ap().rearrange("g p d -> p g d")[:, :g, :],
                    out_acc[:, :g, :])
        transform(*pending)
        nc.scalar.dma_start(
            out_d.ap().rearrange("g p d -> p g d")[:, bpc // 2:, :],
            out_acc[:, bpc // 2:, :])

    nc.compile()
    return nc


def _make_in_maps(x, W, b, pre):
    n, d_in = np.asarray(x).shape
    kt = pre["kt"]
    x16 = np.zeros((kt * P, d_in), ml_dtypes.bfloat16)
    x16[:n] = np.asarray(x, np.float32).astype(ml_dtypes.bfloat16)
    # pre-swizzle to the SBUF layout: x16sw[p, k*d_in+d] = x16[k*P+p, d]
    x16 = np.ascontiguousarray(
        x16.reshape(kt, P, d_in).transpose(1, 0, 2).reshape(P, kt * d_in))
    W32 = np.ascontiguousarray(np.asarray(W, np.float32))
    bias_bcast = np.ascontiguousarray(
        np.tile(np.asarray(b, np.float32)[None, :], (P, 1)))
    ident32 = np.eye(P, dtype=np.float32)
    return [
        dict(x=x16, at=np.ascontiguousarray(pre["at_sw"][c]),
             W=W32, bias=bias_bcast, ident=ident32)
        for c in range(CORES)
    ]


def kernel(x, edge_index, edge_attr, W, b):
    x = np.asarray(x)
    n, d_in = x.shape
    d_out = np.asarray(W).shape[1]
    pre = _preprocess(x, edge_index, edge_attr)
    nc = _build_module(n, d_in, d_out, pre["bpc"], pre["kt"])
    in_maps = _make_in_maps(x, W, b, pre)
    res = run_bass_kernel_spmd(nc, in_maps, list(range(CORES)))
    out_all = np.concatenate([res.results[c]["out"] for c in range(CORES)],
                             axis=0)            # [CORES*bpc, P, d_out]
    out = out_all.reshape(-1, d_out)[:n]
    return np.ascontiguousarray(out.astype(np.float32))



# revision 9
# speedup vs baseline: 4.7500x; 4.7500x over previous
"""GCNConv (PyG semantics: normalize=True, add_self_loops=True, edge_weight)
as a Trainium2 Bass kernel, SPMD over 8 NeuronCores.

Strategy: shard destination nodes across the 8 cores; within a core, process
one 128-dst block at a time. The normalized adjacency (self loops included,
A[dst, src] = dinv[src]*w*dinv[dst]) is sparse: each 128-dst block receives
only ~2.1k edges touching ~2.0k distinct sources, so instead of sweeping all
~10k source columns (79 k-tiles) per block, the host packs each block's
distinct source rows into KT~16 dense k-tiles and emits a matching compacted
A^T stripe. Per block the PE does KT matmuls [128x128 bf16 A'] x [128x512
fp8e3 x'] accumulating agg in PSUM — a ~4.6x FLOP reduction over the dense
sweep at identical numerics on the A side.

The packed x rows are quantized to fp8 e3m4 at 2x scale (4 mantissa bits;
the 1/2 is folded into A'), which keeps the whole-graph working set small
enough (~16MB/core) that both the x stream and the A^T stripes are loaded
into SBUF once in a pipelined prologue and reused; steady state moves only
the output. A' stays bf16 — simulated end-to-end rel-err is 1.3e-2 vs the
2e-2 gate (fp8 on both operands fails). The transform out = agg @ W + b runs
one block behind the sweep in bf16: agg.T is formed by plain matmuls against
identity (cheaper than PE transpose-mode: the 128-col LDWEIGHTS overlaps the
previous matmul), ACT rounds PSUM->SBUF, the W matmuls run at full PE rate,
and DVE adds the bias into a bf16 accumulator that leaves in two DMAs.

All index preprocessing/packing is host-side; every O(E*D)/O(N*D^2) FLOP
runs on device. Per core per pass: ~34us of PE work, which dominates the
~1.3MB of steady-state DMA.
"""
from contextlib import ExitStack

import numpy as np
import ml_dtypes

import concourse.bacc as bacc
import concourse.mybir as mybir
import concourse.tile as tile
from concourse.bass_utils import run_bass_kernel_spmd

P = 128
CORES = 8
BF16 = mybir.dt.bfloat16
F32 = mybir.dt.float32
FP8E3 = mybir.dt.float8e3


def _preprocess(x, edge_index, edge_attr):
    """Self loops, symmetric normalization, then per-block source packing:
    for each 128-dst block, the distinct sources are compacted into KT
    k-tiles. Outputs the packed fp8 x stream and the compacted bf16 A^T
    stripes in SBUF-swizzled layout."""
    x = np.asarray(x, np.float32)
    n, d_in = x.shape
    src = np.asarray(edge_index[0], np.int64)
    dst = np.asarray(edge_index[1], np.int64)
    loop = np.arange(n, dtype=np.int64)
    src_f = np.concatenate([src, loop])
    dst_f = np.concatenate([dst, loop])
    ew = np.concatenate(
        [np.asarray(edge_attr, np.float64), np.ones(n, np.float64)])

    deg = np.zeros(n, np.float64)
    np.add.at(deg, dst_f, ew)
    dinv = np.where(deg > 0, 1.0 / np.sqrt(np.maximum(deg, 1e-300)), 0.0)
    sc = (dinv[src_f] * ew * dinv[dst_f]).astype(np.float32)

    bpc = -(-n // (CORES * P))           # dst blocks per core
    nblocks = CORES * bpc
    blk = dst_f // P
    m_of = dst_f % P

    order = np.argsort(blk, kind="stable")
    blk_s, src_s, m_s, sc_s = blk[order], src_f[order], m_of[order], sc[order]
    bounds = np.searchsorted(blk_s, np.arange(nblocks + 1))
    uniq = []
    slot = np.empty(len(src_s), np.int64)
    for b in range(nblocks):
        lo, hi = bounds[b], bounds[b + 1]
        u, inv = np.unique(src_s[lo:hi], return_inverse=True)
        uniq.append(u)
        slot[lo:hi] = inv
    kt = max(1, max(-(-len(u) // P) for u in uniq))

    at = np.zeros((nblocks, P, kt * P), np.float32)
    np.add.at(at, (blk_s, slot % P, (slot // P) * P + m_s), sc_s)
    at = (at * 0.5).astype(ml_dtypes.bfloat16)     # x carries a 2x scale
    at = at.reshape(CORES, bpc, P, kt * P)

    x2q = (x * 2.0).astype(ml_dtypes.float8_e3m4)
    xq = np.zeros((nblocks, kt * P, d_in), ml_dtypes.float8_e3m4)
    for b in range(nblocks):
        u = uniq[b]
        xq[b, :len(u)] = x2q[u]
    # swizzle to SBUF layout: [p, k*d_in + d] = row k*P+p of the packed block
    xq = (xq.reshape(nblocks, kt, P, d_in).transpose(0, 2, 1, 3)
          .reshape(CORES, bpc, P, kt * d_in))

    return dict(bpc=bpc, kt=kt, at=at, xq=xq)


def _build_module(n, d_in, d_out, bpc, kt, reps=1):
    """Emit the SPMD per-core Bass program."""
    assert d_in % P == 0 and d_out % P == 0
    kt_w = d_in // P

    nc = bacc.Bacc("TRN2", target_bir_lowering=False, debug=False)
    xq_d = nc.dram_tensor("xq", [bpc, P, kt * d_in], FP8E3,
                          kind="ExternalInput")
    at_d = nc.dram_tensor("at", [bpc, P, kt * P], BF16, kind="ExternalInput")
    W_d = nc.dram_tensor("W", [P, kt_w * d_out], BF16, kind="ExternalInput")
    bias_d = nc.dram_tensor("bias", [P, d_out], F32, kind="ExternalInput")
    ident_d = nc.dram_tensor("ident", [P, P], BF16, kind="ExternalInput")
    out_d = nc.dram_tensor("out", [bpc, P, d_out], BF16,
                           kind="ExternalOutput")

    with tile.TileContext(nc) as tc, ExitStack() as ctx:
        const = ctx.enter_context(tc.tile_pool(name="const", bufs=1))
        apool = ctx.enter_context(tc.tile_pool(name="aggsb", bufs=2))
        tpool = ctx.enter_context(tc.tile_pool(name="atsb", bufs=8))
        ps_agg = ctx.enter_context(tc.tile_pool(name="ps_agg", bufs=2,
                                                space="PSUM"))
        ps_t = ctx.enter_context(tc.tile_pool(name="ps_t", bufs=4,
                                              space="PSUM"))
        ps_out = ctx.enter_context(tc.tile_pool(name="ps_out", bufs=2,
                                                space="PSUM"))

        # the whole per-core working set (packed x + A^T stripes) lives in
        # SBUF: loaded once here, consumed by the block loop as each DMA
        # lands (first pass streams through, later reps reuse)
        W_sb = const.tile([P, kt_w, d_out], BF16)
        nc.scalar.dma_start(W_sb[:], W_d.ap().rearrange("p (k d) -> p k d",
                                                        d=d_out))
        bias_sb = const.tile([P, d_out], F32)
        nc.scalar.dma_start(bias_sb[:], bias_d[:, :])
        ident_sb = const.tile([P, P], BF16)
        nc.scalar.dma_start(ident_sb[:], ident_d[:, :])
        out_acc = const.tile([P, bpc, d_out], BF16)
        at_tiles, xq_tiles = [], []
        for g in range(bpc):
            a = const.tile([P, kt, P], BF16, tag=f"at{g}")
            nc.scalar.dma_start(a[:], at_d[g].rearrange("p (k m) -> p k m",
                                                        m=P))
            at_tiles.append(a)
            xx = const.tile([P, kt, d_in], FP8E3, tag=f"xq{g}")
            nc.sync.dma_start(xx[:], xq_d[g].rearrange("p (k d) -> p k d",
                                                       d=d_in))
            xq_tiles.append(xx)

        def transform(g, agg_ps):
            # agg_ps [P dst, d_in] fp32 PSUM -> out_acc[:, g, :] = agg@W + b.
            # agg.T via plain matmuls against identity-free weight loads:
            # lhsT=agg_slice, rhs=W row-block would need agg.T as stationary,
            # so form agg.T slices first, all four ahead of the W matmuls to
            # keep the in-order PE queue from stalling on the ACT relays.
            agg_sb = apool.tile([P, d_in], BF16, tag="agg")
            for ki in range(kt_w):
                # chunked so the first agg.T matmul only waits ~one chunk
                nc.scalar.copy(agg_sb[:, ki * P:(ki + 1) * P],
                               agg_ps[:, ki * P:(ki + 1) * P])
            pts, aTs = [], []
            for ki in range(kt_w):
                pt = ps_t.tile([P, P], F32, tag="pt")
                nc.tensor.matmul(pt[:], agg_sb[:, ki * P:(ki + 1) * P],
                                 ident_sb[:], start=True, stop=True)
                pts.append(pt)
            for ki in range(kt_w):
                aT = tpool.tile([P, P], BF16, tag="aT")
                nc.scalar.copy(aT[:], pts[ki][:])
                aTs.append(aT)
            out_ps = ps_out.tile([P, d_out], F32)
            for ki in range(kt_w):
                nc.tensor.matmul(out_ps[:], aTs[ki][:], W_sb[:, ki, :],
                                 start=(ki == 0), stop=(ki == kt_w - 1))
            nc.vector.tensor_add(out_acc[:, g, :], out_ps[:], bias_sb[:])

        order = [g for _ in range(reps) for g in range(bpc)]
        pending = None              # (g, agg_ps) awaiting transform
        for i, g in enumerate(order):
            at_sb, x_sb = at_tiles[g], xq_tiles[g]
            agg_ps = ps_agg.tile([P, d_in], F32)
            # the previous block's transform is issued two matmuls into this
            # block's sweep so its ACT PSUM->SBUF copy is done by the time
            # the PE reaches the agg.T matmuls
            for k in range(kt):
                nc.tensor.matmul(agg_ps[:], at_sb[:, k, :], x_sb[:, k, :],
                                 start=(k == 0), stop=(k == kt - 1))
                if k == 1 and pending is not None:
                    transform(*pending)
                    pending = None
            if pending is not None:
                transform(*pending)
            pending = (g, agg_ps)
            if g == bpc // 2 and g > 0:
                nc.scalar.dma_start(
                    out_d.ap().rearrange("g p d -> p g d")[:, :g, :],
                    out_acc[:, :g, :])
        transform(*pending)
        nc.scalar.dma_start(
            out_d.ap().rearrange("g p d -> p g d")[:, bpc // 2:, :],
            out_acc[:, bpc // 2:, :])

    nc.compile()
    return nc


def _make_in_maps(x, W, b, pre):
    n, d_in = np.asarray(x).shape
    d_out = np.asarray(W).shape[1]
    kt_w = d_in // P
    W16 = np.ascontiguousarray(
        np.asarray(W, np.float32).astype(ml_dtypes.bfloat16)
        .reshape(kt_w, P, d_out).transpose(1, 0, 2).reshape(P, kt_w * d_out))
    bias_bcast = np.ascontiguousarray(
        np.tile(np.asarray(b, np.float32)[None, :], (P, 1)))
    return [
        dict(xq=np.ascontiguousarray(pre["xq"][c]),
             at=np.ascontiguousarray(pre["at"][c]),
             W=W16, bias=bias_bcast,
             ident=np.eye(P, dtype=ml_dtypes.bfloat16))
        for c in range(CORES)
    ]


def kernel(x, edge_index, edge_attr, W, b):
    x = np.asarray(x)
    n, d_in = x.shape
    d_out = np.asarray(W).shape[1]
    pre = _preprocess(x, edge_index, edge_attr)
    nc = _build_module(n, d_in, d_out, pre["bpc"], pre["kt"])
    in_maps = _make_in_maps(x, W, b, pre)
    res = run_bass_kernel_spmd(nc, in_maps, list(range(CORES)))
    out_all = np.concatenate([res.results[c]["out"] for c in range(CORES)],
                             axis=0)            # [CORES*bpc, P, d_out]
    out = out_all.reshape(-1, d_out)[:n]
    return np.ascontiguousarray(out.astype(np.float32))
